# revision 28
# baseline (speedup 1.0000x reference)
"""Multi-head attention (B=4, N=2048, C=768, H=12) on 8 trn2 NeuronCores.

Sharding: core c handles batch b = c//2 and heads hh = c%2 (6 heads each).
Each core computes Q/K/V for its 6 heads, full attention, and a PARTIAL
output projection; the host sums the two partial projections per batch and
adds the bias.

v5: the ScalarE exp stream (192 x [128,1024] activations, ~1.09us each) is
the pacing floor; everything else is scheduled to keep it fed.
 - Inputs arrive as host-preswizzled SBUF images so every DMA is fully
   contiguous (6-13KB descriptors) across 3 rings (sync/gpsimd/scalar).
 - x.T lives chunk-major ([c-chunk][k][512]) so chunk DMAs are contiguous.
 - Heads run in pairs; scores for the pair interleave row groups (0,0)/
   (64,0); AV keeps the ones-column denominator trick ([V_h|1], 65 cols).
 - All pumped backlog units keep PSUM residency under ~2us so they slot
   into the psa ring between exp reads (pump only AFTER the exps of the
   iteration: ring-reuse WAR stays emission-ordered).
 - Norms are split: DVE reciprocal at kt2/3, PE broadcast+muls at kt5/7.
 - Warm matmul streams bracket the DMA wait and the final norm chain so
   the HAM clock gate never drops the PE to 1.2 GHz mid-kernel.
"""

import os
import sys
from collections import deque

import numpy as np
import ml_dtypes

sys.path.insert(0, "/opt/trn_rl_repo")

import concourse.bass as bass
from concourse import bacc
import concourse.mybir as mybir
from concourse.tile import TileContext
from concourse.bass_utils import run_bass_kernel_spmd
from concourse.dma_utils import dma_copy

P = 128
C = 768
CH = 384             # channels per core (6 heads)
NK = 2048
NQ = 2048
QC = 1024            # query chunk (exp instruction free size)
NH = 6               # local heads
DH = 64
VW = NH * (DH + 1)   # 390
CT = C // P          # 6 contraction tiles
PT = CH // P         # 3 pair tiles
KT = NK // P         # 16 key tiles
XW = CT * 512        # 3072: x image bytes per 512-col chunk
SCALE = DH ** -0.5
F32 = mybir.dt.float32
BF16 = mybir.dt.bfloat16
BF16_NP = ml_dtypes.bfloat16
EXP = mybir.ActivationFunctionType.Exp
COPY = mybir.ActivationFunctionType.Copy

LAST_RESULT = None
_PROG = None


def _build_program() -> bass.Bass:
    nc = bacc.Bacc(None, target_bir_lowering=False)

    # host-preswizzled SBUF images (contiguous DMAs)
    xi = nc.dram_tensor("xi", [P, 4 * XW], BF16, kind="ExternalInput")
    wkqi = nc.dram_tensor("wkqi", [P, 2 * CT * CH], BF16,
                          kind="ExternalInput")
    wvi = nc.dram_tensor("wvi", [P, CT * CH], BF16, kind="ExternalInput")
    wpi = nc.dram_tensor("wpi", [P, PT * C], BF16, kind="ExternalInput")
    y = nc.dram_tensor("y", [NQ, C], BF16, kind="ExternalOutput")
    debug = bool(os.environ.get("BASS_DEBUG_DUMP"))
    if debug:
        dkt = nc.dram_tensor("dkt", [PT * P, NK], F32, kind="ExternalOutput")
        dqt = nc.dram_tensor("dqt", [PT * P, NQ], F32, kind="ExternalOutput")
        dvt = nc.dram_tensor("dvt", [P, KT * VW], F32, kind="ExternalOutput")
        dot = nc.dram_tensor("dot", [PT * P, NQ], F32, kind="ExternalOutput")

    with TileContext(nc) as tc:
        with (
            tc.tile_pool(name="persist", bufs=1) as persist,
            tc.tile_pool(name="pP", bufs=4) as pP,
            tc.tile_pool(name="norm", bufs=4) as nsb,
            tc.tile_pool(name="ysb", bufs=4) as ysb,
            tc.tile_pool(name="psa", bufs=2, space="PSUM") as psa,
            tc.tile_pool(name="psb", bufs=2, space="PSUM") as psb,
        ):
            # ---- ACT table preload + constants
            dum = persist.tile([1, 8], F32, tag="dum")
            nc.gpsimd.memset(dum[:, :], 0.0)
            dumo = persist.tile([1, 8], BF16, tag="dumo")
            nc.scalar.activation(dumo[:, :], dum[:, :], EXP)

            onesb = persist.tile([1, DH], BF16, tag="ones")
            nc.gpsimd.memset(onesb[:, :], 1.0)
            warm = persist.tile([P, 512], BF16, tag="warm")
            nc.gpsimd.memset(warm[:, :], 0.5)
            # keep the PE streaming (HAM warm) while the input DMAs land
            wps = psa.tile([P, QC], F32, tag="a")
            for i in range(100):
                nc.tensor.matmul(wps[:, 0:P], lhsT=warm[:, 0:P],
                                 rhs=warm[:, 0:P], start=True, stop=True)

            # ---- persistent SBUF
            xtb = persist.tile([P, 4 * XW], BF16, tag="xtb", name="xtb")
            wb = persist.tile([P, 3 * CT * CH], BF16, tag="wb", name="wb")
            wpb = persist.tile([P, PT * C], BF16, tag="wpb", name="wpb")
            ktb = [persist.tile([P, NK], BF16, tag=f"kt{i}", name=f"kt{i}")
                   for i in range(PT)]
            qtb = [persist.tile([P, NQ], BF16, tag=f"qt{i}", name=f"qt{i}")
                   for i in range(PT)]
            vtb = persist.tile([P, KT * VW], BF16, tag="vtb", name="vtb")
            otb = [persist.tile([P, NQ], BF16, tag=f"ot{i}", name=f"ot{i}")
                   for i in range(PT)]
            y1p = [persist.tile([P, C], BF16, tag=f"y1_{qt}", name=f"y1_{qt}")
                   for qt in range(QC // P)]

            vones = vtb[:, :].rearrange(
                "p (x e) -> p x e", e=DH + 1)[:, :, DH:DH + 1]
            nc.gpsimd.memset(vones, 1.0)

            # ---- contiguous input DMAs over 3 rings, two phases:
            # critical (x chunks 0,1 + K/Q/V weights) stream first; the
            # rest (x chunks 2,3 + Wp) only start once the first K unit has
            # run, so they don't steal HBM bandwidth from the critical path
            dma_copy(nc.sync, xtb[:, 0:XW], xi[:, 0:XW])
            dma_copy(nc.gpsimd, xtb[:, XW:2 * XW], xi[:, XW:2 * XW])
            dma_copy(nc.scalar, wb[:, 0:2 * CT * CH], wkqi[:, :])
            dma_copy(nc.scalar, wb[:, 2 * CT * CH:3 * CT * CH], wvi[:, :])

            # ---- views (x is chunk-major: [c][k][512])
            def xs(k, col, w):
                b = (col // 512) * XW + k * 512 + col % 512
                return xtb[:, b: b + w]

            def wk_v(k, pair):
                b = k * CH + pair * P
                return wb[:, b: b + P]

            def wq_v(k, pair):
                b = CT * CH + k * CH + pair * P
                return wb[:, b: b + P]

            def wvw(k):
                b = 2 * CT * CH + k * CH
                return wb[:, b: b + CH]

            def vv(kt, hl):
                b = kt * VW + hl * (DH + 1)
                return vtb[:, b: b + DH + 1]

            # ---- work units ----
            def kq_unit(is_k, pair, c0, w=512):
                ps = psa.tile([P, QC], F32, tag="a")
                for k in range(CT):
                    for j in range(0, w, 512):
                        jw = min(512, w - j)
                        nc.tensor.matmul(
                            ps[:, j:j + jw],
                            lhsT=(wk_v if is_k else wq_v)(k, pair),
                            rhs=xs(k, c0 + j, jw),
                            start=(k == 0), stop=(k == CT - 1),
                            skip_group_check=True,
                        )
                dst = (ktb if is_k else qtb)[pair]
                nc.vector.tensor_copy(dst[:, c0:c0 + w], ps[:, 0:w])

            def v_unit(kt):
                ps = psa.tile([P, QC], F32, tag="a")
                for k in range(CT):
                    nc.tensor.matmul(
                        ps[:, 0:CH],
                        lhsT=xs(k, kt * P, P),
                        rhs=wvw(k),
                        start=(k == 0), stop=(k == CT - 1),
                    )
                dst = vtb[:, kt * VW:(kt + 1) * VW].rearrange(
                    "p (h e) -> p h e", e=DH + 1)[:, :, 0:DH]
                src = ps[:, 0:CH].rearrange("p (h e) -> p h e", e=DH)
                nc.vector.tensor_copy(dst, src)

            def proj0(qt):
                q0 = qt * P
                ps = psa.tile([P, QC], F32, tag="a")
                for k in range(PT):
                    for c0, csz in ((0, 512), (512, C - 512)):
                        nc.tensor.matmul(
                            ps[:, c0:c0 + csz],
                            lhsT=otb[k][:, q0:q0 + P],
                            rhs=wpb[:, k * C + c0: k * C + c0 + csz],
                            start=(k == 0), stop=(k == PT - 1),
                            skip_group_check=True,
                        )
                yt = ysb.tile([P, C], BF16, tag="y")
                nc.vector.tensor_copy(yt[:, :], ps[:, 0:C])
                nc.sync.dma_start(out=y[q0:q0 + P, :], in_=yt[:, :])

            def proj_pass1(qt):
                q0 = QC + qt * P
                ps = psa.tile([P, QC], F32, tag="a")
                for k in range(2):
                    for c0, csz in ((0, 512), (512, C - 512)):
                        nc.tensor.matmul(
                            ps[:, c0:c0 + csz],
                            lhsT=otb[k][:, q0:q0 + P],
                            rhs=wpb[:, k * C + c0: k * C + c0 + csz],
                            start=(k == 0), stop=(k == 1),
                            skip_group_check=True,
                        )
                nc.vector.tensor_copy(y1p[qt][:, :], ps[:, 0:C])

            def proj_pass2(qt):
                q0 = QC + qt * P
                ps = psa.tile([P, QC], F32, tag="a")
                for c0, csz in ((0, 512), (512, C - 512)):
                    nc.tensor.matmul(
                        ps[:, c0:c0 + csz],
                        lhsT=otb[2][:, q0:q0 + P],
                        rhs=wpb[:, 2 * C + c0: 2 * C + c0 + csz],
                        start=True, stop=True,
                    )
                yt = ysb.tile([P, C], BF16, tag="y")
                nc.vector.tensor_add(yt[:, :], ps[:, 0:C], y1p[qt][:, :])
                nc.sync.dma_start(out=y[q0:q0 + P, :], in_=yt[:, :])

            backlog = deque()

            def pump(n):
                for _ in range(min(n, len(backlog))):
                    backlog.popleft()()

            # ---- attention pieces ----
            state = {"pend": [], "stash": []}

            def make_av(pt, kt, hl, ot):
                def av():
                    for j in range(2):
                        nc.tensor.matmul(
                            ot[:, j * 512:(j + 1) * 512],
                            lhsT=vv(kt, hl),
                            rhs=pt[:, j * 512:(j + 1) * 512],
                            start=(kt == 0), stop=(kt == KT - 1),
                        )
                return av

            def make_stash(ot, osb, den, use_act=False):
                # use_act: final-pair stash only — ScalarE is idle in the
                # tail, so split the drain across ACT and DVE
                def stash():
                    if use_act:
                        nc.scalar.activation(den[:, :], ot[DH:DH + 1, :],
                                             COPY)
                        nc.scalar.activation(osb[:, :], ot[0:DH, :], COPY)
                    else:
                        nc.vector.tensor_copy(den[:, :], ot[DH:DH + 1, :])
                        nc.vector.tensor_copy(osb[:, :], ot[0:DH, :])
                return stash

            def make_norm(pair, hr, qc, osb, den):
                box = {}

                def pre():
                    rec = nsb.tile([1, QC], F32, tag="rec", bufs=2)
                    nc.vector.reciprocal_approx_fast(out=rec[:, :],
                                                     in_=den[:, :])
                    recb = nsb.tile([1, QC], BF16, tag="recb", bufs=2)
                    nc.vector.tensor_copy(recb[:, :], rec[:, :])
                    box["recb"] = recb

                def mm():
                    recb = box["recb"]
                    rb = psa.tile([P, QC], F32, tag="a")
                    nc.tensor.matmul(
                        rb[0:DH, 0:512], lhsT=onesb[0:1, :],
                        rhs=recb[0:1, 0:512],
                        start=True, stop=True, tile_position=(0, 0),
                    )
                    nc.tensor.matmul(
                        rb[DH:P, 512:QC], lhsT=onesb[0:1, :],
                        rhs=recb[0:1, 512:QC],
                        start=True, stop=True, tile_position=(0, DH),
                    )
                    nc.vector.tensor_mul(
                        otb[pair][hr:hr + DH, qc * QC:qc * QC + 512],
                        osb[:, 0:512], rb[0:DH, 0:512],
                    )
                    nc.vector.tensor_mul(
                        otb[pair][hr:hr + DH, qc * QC + 512:(qc + 1) * QC],
                        osb[:, 512:QC], rb[DH:P, 512:QC],
                    )
                return pre, mm

            # ---- one (qc, pair) block ----
            def block(qc, pair, pump_plan, norms, last=False):
                ots = [psb.tile([DH + 1, QC], F32, tag="b", name=f"ot{hh}")
                       for hh in range(2)]
                for kt in range(KT):
                    sts = []
                    for hh in range(2):
                        st = psa.tile([P, QC], F32, tag="a")
                        sts.append(st)
                    # j-inner: consecutive matmuls share the loaded K tile
                    # (every weight switch costs ~95ns of PE stream)
                    for hh in range(2):
                        hr = hh * DH
                        for j in range(2):
                            nc.tensor.matmul(
                                sts[hh][:, j * 512:(j + 1) * 512],
                                lhsT=ktb[pair][hr:hr + DH,
                                               kt * P:(kt + 1) * P],
                                rhs=qtb[pair][hr:hr + DH,
                                              qc * QC + j * 512:
                                              qc * QC + (j + 1) * 512],
                                start=True, stop=True,
                                tile_position=(hr, 0),
                            )
                    for fn in state["pend"]:
                        fn()
                    state["pend"] = []
                    for fn in state["stash"]:
                        fn()
                    state["stash"] = []
                    for hh in range(2):
                        pt = pP.tile([P, QC], BF16, tag="p")
                        nc.scalar.activation(pt[:, :], sts[hh][:, :], EXP,
                                             scale=SCALE)
                        state["pend"].append(
                            make_av(pt, kt, pair * 2 + hh, ots[hh]))
                    # pump/norm AFTER the exps (psa ring-reuse WAR order)
                    if kt == 2 and norms:
                        norms[0][0]()
                    elif kt == 3 and norms:
                        norms[1][0]()
                    elif kt == 5 and norms:
                        norms[0][1]()
                    elif kt == 7 and norms:
                        norms[1][1]()
                    else:
                        pump(pump_plan[kt])
                out_norms = []
                for hh in range(2):
                    osb = nsb.tile([DH, QC], F32, tag="osb", bufs=4,
                                   name=f"osb{hh}")
                    den = nsb.tile([1, QC], F32, tag="den", bufs=4,
                                   name=f"den{hh}")
                    state["stash"].append(
                        make_stash(ots[hh], osb, den, use_act=bool(hh)))
                    out_norms.append(make_norm(pair, hh * DH, qc, osb, den))
                return out_norms

            # ---- prelude compute: minimum for the first exp ----
            kq_unit(True, 0, 0, 128)     # K pair0 keys 0-127
            # deferred-DMA trigger: a tiny gpsimd copy that depends on the
            # first K unit gates the non-critical DMAs behind it
            trig = persist.tile([1, 8], BF16, tag="trig")
            nc.gpsimd.tensor_copy(trig[:, :], ktb[0][0:1, 0:8])
            for c in (2, 3):
                dma_copy(nc.gpsimd, xtb[:, c * XW:(c + 1) * XW],
                         xi[:, c * XW:(c + 1) * XW])
            dma_copy(nc.gpsimd, wpb[:, :], wpi[:, :])
            kq_unit(False, 0, 0, 1024)   # Q pair0 queries 0-1023
            kq_unit(True, 0, 512)        # K pair0 keys 512-1023 (chunk 1)

            # ---- backlog, dependency order; V(kt) just-in-time ----
            def V(kt):
                backlog.append(lambda kt=kt: v_unit(kt))

            def KQ(is_k, pair, c0, w=512):
                backlog.append(
                    lambda i=is_k, p=pair, c=c0, w=w: kq_unit(i, p, c, w))

            KQ(True, 0, 128, 384)
            V(0); V(1); V(2); V(3); V(4)
            KQ(True, 0, 1024); V(5); V(6)
            KQ(True, 0, 1536); V(7); V(8)
            KQ(False, 0, 1024); V(9)
            KQ(False, 0, 1536)
            for kt in range(10, KT):
                V(kt)
            for c in range(4):
                KQ(True, 1, c * 512)
            for c in range(4):
                KQ(False, 1, c * 512)
            for c in range(4):
                KQ(True, 2, c * 512)
            for c in range(4):
                KQ(False, 2, c * 512)

            # ---- blocks, pair-major; block b runs block b-1's norms ----
            plan0 = [2, 2, 2, 1, 1, 2, 1, 2, 1, 2, 2, 1, 1, 1, 1, 1]
            plan_b1 = [1, 1, 0, 0, 1, 0, 1, 0, 1, 1, 0, 0, 0, 0, 0, 0]
            plan_b2 = [1, 1, 0, 0, 1, 0, 1, 0, 0, 0, 0, 0, 0, 0, 0, 0]
            plan_late = [0] * 8 + [1] * 8
            nrm = block(0, 0, plan0, None)
            nrm = block(1, 0, plan_b1, nrm)
            nrm = block(0, 1, plan_b2, nrm)
            nrm = block(1, 1, plan_b1, nrm)
            for qt in range(QC // P):
                backlog.append(lambda qt=qt: proj_pass1(qt))
            nrm = block(0, 2, plan_late, nrm)
            for qt in range(QC // P):
                backlog.append(lambda qt=qt: proj0(qt))
            nrm = block(1, 2, plan_late, nrm, last=True)

            # ---- tail ----
            for fn in state["pend"]:
                fn()
            state["pend"] = []
            for fn in state["stash"]:
                fn()
            state["stash"] = []
            pump(len(backlog))
            # keep the PE warm through the final norm's DVE chain
            wtl = psa.tile([P, QC], F32, tag="a")
            for i in range(65):
                nc.tensor.matmul(wtl[:, 0:P], lhsT=warm[:, 0:P],
                                 rhs=warm[:, 0:P], start=True, stop=True)
            nrm[0][0]()
            nrm[1][0]()
            nrm[0][1]()
            nrm[1][1]()
            for qt in range(QC // P):
                proj_pass2(qt)
            if debug:
                for i in range(PT):
                    tmp = ysb.tile([P, NK], F32, tag="dbg", bufs=2)
                    nc.vector.tensor_copy(tmp[:, :], ktb[i][:, :])
                    nc.sync.dma_start(out=dkt[i * P:(i + 1) * P, :],
                                      in_=tmp[:, :])
                    tmp = ysb.tile([P, NQ], F32, tag="dbg", bufs=2)
                    nc.vector.tensor_copy(tmp[:, :], qtb[i][:, :])
                    nc.sync.dma_start(out=dqt[i * P:(i + 1) * P, :],
                                      in_=tmp[:, :])
                    tmp = ysb.tile([P, NQ], F32, tag="dbg", bufs=2)
                    nc.vector.tensor_copy(tmp[:, :], otb[i][:, :])
                    nc.sync.dma_start(out=dot[i * P:(i + 1) * P, :],
                                      in_=tmp[:, :])
                tmp = ysb.tile([P, KT * VW], F32, tag="dbgv", bufs=1)
                nc.vector.tensor_copy(tmp[:, :], vtb[:, :])
                nc.sync.dma_start(out=dvt[:, :], in_=tmp[:, :])

    nc.compile()
    return nc


def _get_prog() -> bass.Bass:
    global _PROG
    if _PROG is None:
        _PROG = _build_program()
    return _PROG


def _swizzle(mat, inner):
    # [CT*128, inner] channel-major -> [128, CT*inner] SBUF image
    kt = mat.shape[0] // P
    return np.ascontiguousarray(
        mat.reshape(kt, P, inner).transpose(1, 0, 2).reshape(P, kt * inner))


def kernel(x, Wq, Wk, Wv, Wp, bp):
    global LAST_RESULT
    x = np.asarray(x, np.float32)
    Wq = np.asarray(Wq, np.float32)
    Wk = np.asarray(Wk, np.float32)
    Wv = np.asarray(Wv, np.float32)
    Wp = np.asarray(Wp, np.float32)
    bp = np.asarray(bp, np.float32)

    B, N, _ = x.shape
    # x image: [p, c-chunk, k, 512] chunk-major flat
    xis = []
    for b in range(B):
        xT = np.ascontiguousarray(x[b].T)              # [768, 2048]
        img = xT.reshape(CT, P, 4, 512).transpose(1, 2, 0, 3)
        xis.append(np.ascontiguousarray(
            img.reshape(P, 4 * XW)).astype(BF16_NP))
    wkq_h, wv_h, wp_h = [], [], []
    for hh in range(2):
        r = slice(hh * CH, (hh + 1) * CH)
        kimg = _swizzle(Wk[r].T, CH)
        qimg = _swizzle(Wq[r].T, CH)
        vimg = _swizzle(Wv[r].T, CH)
        wkq_h.append(np.concatenate([kimg, qimg], axis=1).astype(BF16_NP))
        wv_h.append(vimg.astype(BF16_NP))
        wp_h.append(_swizzle(Wp.T[r], C).astype(BF16_NP))

    in_maps = []
    for core in range(8):
        b, hh = core // 2, core % 2
        in_maps.append({
            "xi": xis[b],
            "wkqi": wkq_h[hh],
            "wvi": wv_h[hh],
            "wpi": wp_h[hh],
        })

    res = run_bass_kernel_spmd(
        _get_prog(), in_maps, core_ids=list(range(8)),
        trace=bool(os.environ.get("BASS_TRACE")),
    )
    LAST_RESULT = res

    out = np.empty((B, N, C), np.float32)
    for b in range(B):
        out[b] = (res.results[2 * b]["y"].astype(np.float32)
                  + res.results[2 * b + 1]["y"].astype(np.float32) + bp)
    return out


# revision 29
# speedup vs baseline: 1.2289x; 1.2289x over previous
"""Multi-head attention (B=4, N=2048, C=768, H=12) on 8 trn2 NeuronCores.

Sharding: core c handles batch b = c//2 and heads hh = c%2 (6 heads each).
Each core computes Q/K/V for its 6 heads, full attention, and a PARTIAL
output projection; the host sums the two partial projections per batch and
adds the bias.

v5: the ScalarE exp stream (192 x [128,1024] activations, ~1.09us each) is
the pacing floor; everything else is scheduled to keep it fed.
 - Inputs arrive as host-preswizzled SBUF images so every DMA is fully
   contiguous (6-13KB descriptors) across 3 rings (sync/gpsimd/scalar).
 - x.T lives chunk-major ([c-chunk][k][512]) so chunk DMAs are contiguous.
 - Heads run in pairs; scores for the pair interleave row groups (0,0)/
   (64,0); AV keeps the ones-column denominator trick ([V_h|1], 65 cols).
 - All pumped backlog units keep PSUM residency under ~2us so they slot
   into the psa ring between exp reads (pump only AFTER the exps of the
   iteration: ring-reuse WAR stays emission-ordered).
 - Norms are split: DVE reciprocal at kt2/3, PE broadcast+muls at kt5/7.
 - Warm matmul streams bracket the DMA wait and the final norm chain so
   the HAM clock gate never drops the PE to 1.2 GHz mid-kernel.
"""

import os
import sys
from collections import deque

import numpy as np
import ml_dtypes

sys.path.insert(0, "/opt/trn_rl_repo")

import concourse.bass as bass
from concourse import bacc
import concourse.mybir as mybir
from concourse.tile import TileContext
from concourse.bass_utils import run_bass_kernel_spmd
from concourse.dma_utils import dma_copy

P = 128
C = 768
CH = 384             # channels per core (6 heads)
NK = 2048
NQ = 2048
QC = 1024            # query chunk (exp instruction free size)
NH = 6               # local heads
DH = 64
VW = NH * (DH + 1)   # 390
CT = C // P          # 6 contraction tiles
PT = CH // P         # 3 pair tiles
KT = NK // P         # 16 key tiles
XW = CT * 512        # 3072: x image bytes per 512-col chunk
SCALE = DH ** -0.5
F32 = mybir.dt.float32
BF16 = mybir.dt.bfloat16
BF16_NP = ml_dtypes.bfloat16
EXP = mybir.ActivationFunctionType.Exp
COPY = mybir.ActivationFunctionType.Copy

LAST_RESULT = None
_PROG = None


def _build_program() -> bass.Bass:
    nc = bacc.Bacc(None, target_bir_lowering=False)

    # host-preswizzled SBUF images (contiguous DMAs)
    xi = nc.dram_tensor("xi", [P, 4 * XW], BF16, kind="ExternalInput")
    wkqi = nc.dram_tensor("wkqi", [P, 2 * CT * CH], BF16,
                          kind="ExternalInput")
    wvi = nc.dram_tensor("wvi", [P, CT * CH], BF16, kind="ExternalInput")
    wpi = nc.dram_tensor("wpi", [P, PT * C], BF16, kind="ExternalInput")
    y = nc.dram_tensor("y", [NQ, C], BF16, kind="ExternalOutput")
    debug = bool(os.environ.get("BASS_DEBUG_DUMP"))
    if debug:
        dkt = nc.dram_tensor("dkt", [PT * P, NK], F32, kind="ExternalOutput")
        dqt = nc.dram_tensor("dqt", [PT * P, NQ], F32, kind="ExternalOutput")
        dvt = nc.dram_tensor("dvt", [P, KT * VW], F32, kind="ExternalOutput")
        dot = nc.dram_tensor("dot", [PT * P, NQ], F32, kind="ExternalOutput")

    with TileContext(nc) as tc:
        with (
            tc.tile_pool(name="persist", bufs=1) as persist,
            tc.tile_pool(name="pP", bufs=4) as pP,
            tc.tile_pool(name="norm", bufs=4) as nsb,
            tc.tile_pool(name="ysb", bufs=4) as ysb,
            tc.tile_pool(name="psa", bufs=2, space="PSUM") as psa,
            tc.tile_pool(name="psb", bufs=2, space="PSUM") as psb,
        ):
            # ---- ACT table preload + constants
            dum = persist.tile([1, 8], F32, tag="dum")
            nc.gpsimd.memset(dum[:, :], 0.0)
            dumo = persist.tile([1, 8], BF16, tag="dumo")
            nc.scalar.activation(dumo[:, :], dum[:, :], EXP)

            onesb = persist.tile([1, DH], BF16, tag="ones")
            nc.gpsimd.memset(onesb[:, :], 1.0)
            warm = persist.tile([P, 512], BF16, tag="warm")
            nc.gpsimd.memset(warm[:, :], 0.5)
            # keep the PE streaming (HAM warm) while the input DMAs land
            wps = psa.tile([P, QC], F32, tag="a")
            for i in range(60):
                nc.tensor.matmul(wps[:, 0:P], lhsT=warm[:, 0:P],
                                 rhs=warm[:, 0:P], start=True, stop=True)

            # ---- persistent SBUF
            xtb = persist.tile([P, 4 * XW], BF16, tag="xtb", name="xtb")
            wb = persist.tile([P, 3 * CT * CH], BF16, tag="wb", name="wb")
            wpb = persist.tile([P, PT * C], BF16, tag="wpb", name="wpb")
            ktb = [persist.tile([P, NK], BF16, tag=f"kt{i}", name=f"kt{i}")
                   for i in range(PT)]
            qtb = [persist.tile([P, NQ], BF16, tag=f"qt{i}", name=f"qt{i}")
                   for i in range(PT)]
            vtb = persist.tile([P, KT * VW], BF16, tag="vtb", name="vtb")
            otb = [persist.tile([P, NQ], BF16, tag=f"ot{i}", name=f"ot{i}")
                   for i in range(PT)]
            y1p = [persist.tile([P, C], BF16, tag=f"y1_{qt}", name=f"y1_{qt}")
                   for qt in range(QC // P)]

            vones = vtb[:, :].rearrange(
                "p (x e) -> p x e", e=DH + 1)[:, :, DH:DH + 1]
            nc.gpsimd.memset(vones, 1.0)

            # ---- contiguous input DMAs over 3 rings, two phases:
            # critical (x chunks 0,1 + K/Q/V weights) stream first; the
            # rest (x chunks 2,3 + Wp) only start once the first K unit has
            # run, so they don't steal HBM bandwidth from the critical path
            dma_copy(nc.sync, xtb[:, 0:XW], xi[:, 0:XW])
            dma_copy(nc.gpsimd, xtb[:, XW:2 * XW], xi[:, XW:2 * XW])
            dma_copy(nc.scalar, wb[:, 0:2 * CT * CH], wkqi[:, :])
            dma_copy(nc.scalar, wb[:, 2 * CT * CH:3 * CT * CH], wvi[:, :])

            # ---- views (x is chunk-major: [c][k][512])
            def xs(k, col, w):
                b = (col // 512) * XW + k * 512 + col % 512
                return xtb[:, b: b + w]

            def wk_v(k, pair):
                b = k * CH + pair * P
                return wb[:, b: b + P]

            def wq_v(k, pair):
                b = CT * CH + k * CH + pair * P
                return wb[:, b: b + P]

            def wvw(k):
                b = 2 * CT * CH + k * CH
                return wb[:, b: b + CH]

            def vv(kt, hl):
                b = kt * VW + hl * (DH + 1)
                return vtb[:, b: b + DH + 1]

            # ---- work units ----
            def kq_unit(is_k, pair, c0, w=512):
                ps = psa.tile([P, QC], F32, tag="a")
                for k in range(CT):
                    for j in range(0, w, 512):
                        jw = min(512, w - j)
                        nc.tensor.matmul(
                            ps[:, j:j + jw],
                            lhsT=(wk_v if is_k else wq_v)(k, pair),
                            rhs=xs(k, c0 + j, jw),
                            start=(k == 0), stop=(k == CT - 1),
                            skip_group_check=True,
                        )
                dst = (ktb if is_k else qtb)[pair]
                nc.vector.tensor_copy(dst[:, c0:c0 + w], ps[:, 0:w])

            def v_unit(kt):
                ps = psa.tile([P, QC], F32, tag="a")
                for k in range(CT):
                    nc.tensor.matmul(
                        ps[:, 0:CH],
                        lhsT=xs(k, kt * P, P),
                        rhs=wvw(k),
                        start=(k == 0), stop=(k == CT - 1),
                    )
                dst = vtb[:, kt * VW:(kt + 1) * VW].rearrange(
                    "p (h e) -> p h e", e=DH + 1)[:, :, 0:DH]
                src = ps[:, 0:CH].rearrange("p (h e) -> p h e", e=DH)
                nc.vector.tensor_copy(dst, src)

            def proj0(qt):
                q0 = qt * P
                ps = psa.tile([P, QC], F32, tag="a")
                for k in range(PT):
                    for c0, csz in ((0, 512), (512, C - 512)):
                        nc.tensor.matmul(
                            ps[:, c0:c0 + csz],
                            lhsT=otb[k][:, q0:q0 + P],
                            rhs=wpb[:, k * C + c0: k * C + c0 + csz],
                            start=(k == 0), stop=(k == PT - 1),
                            skip_group_check=True,
                        )
                yt = ysb.tile([P, C], BF16, tag="y")
                nc.vector.tensor_copy(yt[:, :], ps[:, 0:C])
                nc.sync.dma_start(out=y[q0:q0 + P, :], in_=yt[:, :])

            def proj_pass1(qt):
                q0 = QC + qt * P
                ps = psa.tile([P, QC], F32, tag="a")
                for k in range(2):
                    for c0, csz in ((0, 512), (512, C - 512)):
                        nc.tensor.matmul(
                            ps[:, c0:c0 + csz],
                            lhsT=otb[k][:, q0:q0 + P],
                            rhs=wpb[:, k * C + c0: k * C + c0 + csz],
                            start=(k == 0), stop=(k == 1),
                            skip_group_check=True,
                        )
                nc.vector.tensor_copy(y1p[qt][:, :], ps[:, 0:C])

            def proj_pass2(qt):
                q0 = QC + qt * P
                ps = psa.tile([P, QC], F32, tag="a")
                for c0, csz in ((0, 512), (512, C - 512)):
                    nc.tensor.matmul(
                        ps[:, c0:c0 + csz],
                        lhsT=otb[2][:, q0:q0 + P],
                        rhs=wpb[:, 2 * C + c0: 2 * C + c0 + csz],
                        start=True, stop=True,
                    )
                yt = ysb.tile([P, C], BF16, tag="y")
                nc.vector.tensor_add(yt[:, :], ps[:, 0:C], y1p[qt][:, :])
                nc.sync.dma_start(out=y[q0:q0 + P, :], in_=yt[:, :])

            backlog = deque()

            def pump(n):
                for _ in range(min(n, len(backlog))):
                    backlog.popleft()()

            # ---- attention pieces ----
            state = {"pend": [], "stash": []}

            def make_av(pt, kt, hl, ot):
                def av():
                    for j in range(2):
                        nc.tensor.matmul(
                            ot[:, j * 512:(j + 1) * 512],
                            lhsT=vv(kt, hl),
                            rhs=pt[:, j * 512:(j + 1) * 512],
                            start=(kt == 0), stop=(kt == KT - 1),
                        )
                return av

            def make_stash(ot, osb, den, use_act=False):
                # use_act: final-pair stash only — ScalarE is idle in the
                # tail, so split the drain across ACT and DVE
                def stash():
                    if use_act:
                        nc.scalar.activation(den[:, :], ot[DH:DH + 1, :],
                                             COPY)
                        nc.scalar.activation(osb[:, :], ot[0:DH, :], COPY)
                    else:
                        nc.vector.tensor_copy(den[:, :], ot[DH:DH + 1, :])
                        nc.vector.tensor_copy(osb[:, :], ot[0:DH, :])
                return stash

            def make_norm(pair, hr, qc, osb, den):
                box = {}

                def pre():
                    rec = nsb.tile([1, QC], F32, tag="rec", bufs=2)
                    nc.vector.reciprocal_approx_fast(out=rec[:, :],
                                                     in_=den[:, :])
                    recb = nsb.tile([1, QC], BF16, tag="recb", bufs=2)
                    nc.vector.tensor_copy(recb[:, :], rec[:, :])
                    box["recb"] = recb

                def mm():
                    recb = box["recb"]
                    rb = psa.tile([P, QC], F32, tag="a")
                    nc.tensor.matmul(
                        rb[0:DH, 0:512], lhsT=onesb[0:1, :],
                        rhs=recb[0:1, 0:512],
                        start=True, stop=True, tile_position=(0, 0),
                    )
                    nc.tensor.matmul(
                        rb[DH:P, 512:QC], lhsT=onesb[0:1, :],
                        rhs=recb[0:1, 512:QC],
                        start=True, stop=True, tile_position=(0, DH),
                    )
                    nc.vector.tensor_mul(
                        otb[pair][hr:hr + DH, qc * QC:qc * QC + 512],
                        osb[:, 0:512], rb[0:DH, 0:512],
                    )
                    nc.vector.tensor_mul(
                        otb[pair][hr:hr + DH, qc * QC + 512:(qc + 1) * QC],
                        osb[:, 512:QC], rb[DH:P, 512:QC],
                    )
                return pre, mm

            # ---- one (qc, pair) block ----
            def block(qc, pair, pump_plan, norms, last=False):
                ots = [psb.tile([DH + 1, QC], F32, tag="b", name=f"ot{hh}")
                       for hh in range(2)]
                for kt in range(KT):
                    sts = []
                    for hh in range(2):
                        st = psa.tile([P, QC], F32, tag="a")
                        sts.append(st)
                    # j-inner: consecutive matmuls share the loaded K tile
                    # (every weight switch costs ~95ns of PE stream)
                    for hh in range(2):
                        hr = hh * DH
                        for j in range(2):
                            nc.tensor.matmul(
                                sts[hh][:, j * 512:(j + 1) * 512],
                                lhsT=ktb[pair][hr:hr + DH,
                                               kt * P:(kt + 1) * P],
                                rhs=qtb[pair][hr:hr + DH,
                                              qc * QC + j * 512:
                                              qc * QC + (j + 1) * 512],
                                start=True, stop=True,
                                tile_position=(hr, 0),
                            )
                    for fn in state["pend"]:
                        fn()
                    state["pend"] = []
                    for fn in state["stash"]:
                        fn()
                    state["stash"] = []
                    for hh in range(2):
                        pt = pP.tile([P, QC], BF16, tag="p")
                        nc.scalar.activation(pt[:, :], sts[hh][:, :], EXP,
                                             scale=SCALE)
                        state["pend"].append(
                            make_av(pt, kt, pair * 2 + hh, ots[hh]))
                    # pump/norm AFTER the exps (psa ring-reuse WAR order)
                    if kt == 2 and norms:
                        norms[0][0]()
                    elif kt == 3 and norms:
                        norms[1][0]()
                    elif kt == 5 and norms:
                        norms[0][1]()
                    elif kt == 7 and norms:
                        norms[1][1]()
                    else:
                        pump(pump_plan[kt])
                out_norms = []
                for hh in range(2):
                    osb = nsb.tile([DH, QC], F32, tag="osb", bufs=4,
                                   name=f"osb{hh}")
                    den = nsb.tile([1, QC], F32, tag="den", bufs=4,
                                   name=f"den{hh}")
                    state["stash"].append(
                        make_stash(ots[hh], osb, den, use_act=(last and hh)))
                    out_norms.append(make_norm(pair, hh * DH, qc, osb, den))
                return out_norms

            # ---- prelude compute: minimum for the first exp ----
            kq_unit(True, 0, 0, 128)     # K pair0 keys 0-127
            # deferred-DMA trigger: a tiny gpsimd copy that depends on the
            # first K unit gates the non-critical DMAs behind it
            trig = persist.tile([1, 8], BF16, tag="trig")
            nc.gpsimd.tensor_copy(trig[:, :], ktb[0][0:1, 0:8])
            for c in (2, 3):
                dma_copy(nc.gpsimd, xtb[:, c * XW:(c + 1) * XW],
                         xi[:, c * XW:(c + 1) * XW])
            dma_copy(nc.gpsimd, wpb[:, :], wpi[:, :])
            kq_unit(False, 0, 0, 1024)   # Q pair0 queries 0-1023
            kq_unit(True, 0, 512)        # K pair0 keys 512-1023 (chunk 1)

            # ---- backlog, dependency order; V(kt) just-in-time ----
            def V(kt):
                backlog.append(lambda kt=kt: v_unit(kt))

            def KQ(is_k, pair, c0, w=512):
                backlog.append(
                    lambda i=is_k, p=pair, c=c0, w=w: kq_unit(i, p, c, w))

            KQ(True, 0, 128, 384)
            V(0); V(1); V(2); V(3); V(4)
            KQ(True, 0, 1024); V(5); V(6)
            KQ(True, 0, 1536); V(7); V(8)
            KQ(False, 0, 1024); V(9)
            KQ(False, 0, 1536)
            for kt in range(10, KT):
                V(kt)
            for c in range(4):
                KQ(True, 1, c * 512)
            for c in range(4):
                KQ(False, 1, c * 512)
            for c in range(4):
                KQ(True, 2, c * 512)
            for c in range(4):
                KQ(False, 2, c * 512)

            # ---- blocks, pair-major; block b runs block b-1's norms ----
            plan0 = [2, 2, 2, 1, 1, 2, 1, 2, 1, 2, 2, 1, 1, 1, 1, 1]
            plan_b1 = [1, 1, 0, 0, 1, 0, 1, 0, 1, 1, 0, 0, 0, 0, 0, 0]
            plan_b2 = [1, 1, 0, 0, 1, 0, 1, 0, 0, 0, 0, 0, 0, 0, 0, 0]
            plan_late = [0] * 8 + [1] * 8
            nrm = block(0, 0, plan0, None)
            nrm = block(1, 0, plan_b1, nrm)
            nrm = block(0, 1, plan_b2, nrm)
            nrm = block(1, 1, plan_b1, nrm)
            for qt in range(QC // P):
                backlog.append(lambda qt=qt: proj_pass1(qt))
            nrm = block(0, 2, plan_late, nrm)
            for qt in range(QC // P):
                backlog.append(lambda qt=qt: proj0(qt))
            nrm = block(1, 2, plan_late, nrm, last=True)

            # ---- tail ----
            for fn in state["pend"]:
                fn()
            state["pend"] = []
            for fn in state["stash"]:
                fn()
            state["stash"] = []
            pump(len(backlog))
            # keep the PE warm through the final norm's DVE chain
            wtl = psa.tile([P, QC], F32, tag="a")
            for i in range(45):
                nc.tensor.matmul(wtl[:, 0:P], lhsT=warm[:, 0:P],
                                 rhs=warm[:, 0:P], start=True, stop=True)
            nrm[0][0]()
            nrm[1][0]()
            nrm[0][1]()
            nrm[1][1]()
            for qt in range(QC // P):
                proj_pass2(qt)
            if debug:
                for i in range(PT):
                    tmp = ysb.tile([P, NK], F32, tag="dbg", bufs=2)
                    nc.vector.tensor_copy(tmp[:, :], ktb[i][:, :])
                    nc.sync.dma_start(out=dkt[i * P:(i + 1) * P, :],
                                      in_=tmp[:, :])
                    tmp = ysb.tile([P, NQ], F32, tag="dbg", bufs=2)
                    nc.vector.tensor_copy(tmp[:, :], qtb[i][:, :])
                    nc.sync.dma_start(out=dqt[i * P:(i + 1) * P, :],
                                      in_=tmp[:, :])
                    tmp = ysb.tile([P, NQ], F32, tag="dbg", bufs=2)
                    nc.vector.tensor_copy(tmp[:, :], otb[i][:, :])
                    nc.sync.dma_start(out=dot[i * P:(i + 1) * P, :],
                                      in_=tmp[:, :])
                tmp = ysb.tile([P, KT * VW], F32, tag="dbgv", bufs=1)
                nc.vector.tensor_copy(tmp[:, :], vtb[:, :])
                nc.sync.dma_start(out=dvt[:, :], in_=tmp[:, :])

    nc.compile()
    return nc


def _get_prog() -> bass.Bass:
    global _PROG
    if _PROG is None:
        _PROG = _build_program()
    return _PROG


def _swizzle(mat, inner):
    # [CT*128, inner] channel-major -> [128, CT*inner] SBUF image
    kt = mat.shape[0] // P
    return np.ascontiguousarray(
        mat.reshape(kt, P, inner).transpose(1, 0, 2).reshape(P, kt * inner))


def kernel(x, Wq, Wk, Wv, Wp, bp):
    global LAST_RESULT
    x = np.asarray(x, np.float32)
    Wq = np.asarray(Wq, np.float32)
    Wk = np.asarray(Wk, np.float32)
    Wv = np.asarray(Wv, np.float32)
    Wp = np.asarray(Wp, np.float32)
    bp = np.asarray(bp, np.float32)

    B, N, _ = x.shape
    # x image: [p, c-chunk, k, 512] chunk-major flat
    xis = []
    for b in range(B):
        xT = np.ascontiguousarray(x[b].T)              # [768, 2048]
        img = xT.reshape(CT, P, 4, 512).transpose(1, 2, 0, 3)
        xis.append(np.ascontiguousarray(
            img.reshape(P, 4 * XW)).astype(BF16_NP))
    wkq_h, wv_h, wp_h = [], [], []
    for hh in range(2):
        r = slice(hh * CH, (hh + 1) * CH)
        kimg = _swizzle(Wk[r].T, CH)
        qimg = _swizzle(Wq[r].T, CH)
        vimg = _swizzle(Wv[r].T, CH)
        wkq_h.append(np.concatenate([kimg, qimg], axis=1).astype(BF16_NP))
        wv_h.append(vimg.astype(BF16_NP))
        wp_h.append(_swizzle(Wp.T[r], C).astype(BF16_NP))

    in_maps = []
    for core in range(8):
        b, hh = core // 2, core % 2
        in_maps.append({
            "xi": xis[b],
            "wkqi": wkq_h[hh],
            "wvi": wv_h[hh],
            "wpi": wp_h[hh],
        })

    res = run_bass_kernel_spmd(
        _get_prog(), in_maps, core_ids=list(range(8)),
        trace=bool(os.environ.get("BASS_TRACE")),
    )
    LAST_RESULT = res

    out = np.empty((B, N, C), np.float32)
    for b in range(B):
        out[b] = (res.results[2 * b]["y"].astype(np.float32)
                  + res.results[2 * b + 1]["y"].astype(np.float32) + bp)
    return out


# revision 30
# speedup vs baseline: 1.2386x; 1.0079x over previous
"""Multi-head attention (B=4, N=2048, C=768, H=12) on 8 trn2 NeuronCores.

Sharding: core c handles batch b = c//2 and heads hh = c%2 (6 heads each).
Each core computes Q/K/V for its 6 heads, full attention, and a PARTIAL
output projection; the host sums the two partial projections per batch and
adds the bias.

v5: the ScalarE exp stream (192 x [128,1024] activations, ~1.09us each) is
the pacing floor; everything else is scheduled to keep it fed.
 - Inputs arrive as host-preswizzled SBUF images so every DMA is fully
   contiguous (6-13KB descriptors) across 3 rings (sync/gpsimd/scalar).
 - x.T lives chunk-major ([c-chunk][k][512]) so chunk DMAs are contiguous.
 - Heads run in pairs; scores for the pair interleave row groups (0,0)/
   (64,0); AV keeps the ones-column denominator trick ([V_h|1], 65 cols).
 - All pumped backlog units keep PSUM residency under ~2us so they slot
   into the psa ring between exp reads (pump only AFTER the exps of the
   iteration: ring-reuse WAR stays emission-ordered).
 - Norms are split: DVE reciprocal at kt2/3, PE broadcast+muls at kt5/7.
 - Warm matmul streams bracket the DMA wait and the final norm chain so
   the HAM clock gate never drops the PE to 1.2 GHz mid-kernel.
"""

import os
import sys
from collections import deque

import numpy as np
import ml_dtypes

sys.path.insert(0, "/opt/trn_rl_repo")

import concourse.bass as bass
from concourse import bacc
import concourse.mybir as mybir
from concourse.tile import TileContext
from concourse.bass_utils import run_bass_kernel_spmd
from concourse.dma_utils import dma_copy

P = 128
C = 768
CH = 384             # channels per core (6 heads)
NK = 2048
NQ = 2048
QC = 1024            # query chunk (exp instruction free size)
NH = 6               # local heads
DH = 64
VW = NH * (DH + 1)   # 390
CT = C // P          # 6 contraction tiles
PT = CH // P         # 3 pair tiles
KT = NK // P         # 16 key tiles
XW = CT * 512        # 3072: x image bytes per 512-col chunk
SCALE = DH ** -0.5
F32 = mybir.dt.float32
BF16 = mybir.dt.bfloat16
BF16_NP = ml_dtypes.bfloat16
EXP = mybir.ActivationFunctionType.Exp
COPY = mybir.ActivationFunctionType.Copy

LAST_RESULT = None
_PROG = None


def _build_program() -> bass.Bass:
    nc = bacc.Bacc(None, target_bir_lowering=False)

    # host-preswizzled SBUF images (contiguous DMAs)
    xi = nc.dram_tensor("xi", [P, 4 * XW], BF16, kind="ExternalInput")
    wkqi = nc.dram_tensor("wkqi", [P, 2 * CT * CH], BF16,
                          kind="ExternalInput")
    wvi = nc.dram_tensor("wvi", [P, CT * CH], BF16, kind="ExternalInput")
    wpi = nc.dram_tensor("wpi", [P, PT * C], BF16, kind="ExternalInput")
    y = nc.dram_tensor("y", [NQ, C], BF16, kind="ExternalOutput")
    debug = bool(os.environ.get("BASS_DEBUG_DUMP"))
    if debug:
        dkt = nc.dram_tensor("dkt", [PT * P, NK], F32, kind="ExternalOutput")
        dqt = nc.dram_tensor("dqt", [PT * P, NQ], F32, kind="ExternalOutput")
        dvt = nc.dram_tensor("dvt", [P, KT * VW], F32, kind="ExternalOutput")
        dot = nc.dram_tensor("dot", [PT * P, NQ], F32, kind="ExternalOutput")

    with TileContext(nc) as tc:
        with (
            tc.tile_pool(name="persist", bufs=1) as persist,
            tc.tile_pool(name="pP", bufs=4) as pP,
            tc.tile_pool(name="norm", bufs=4) as nsb,
            tc.tile_pool(name="ysb", bufs=4) as ysb,
            tc.tile_pool(name="psa", bufs=2, space="PSUM") as psa,
            tc.tile_pool(name="psb", bufs=2, space="PSUM") as psb,
        ):
            # ---- ACT table preload + constants
            dum = persist.tile([1, 8], F32, tag="dum")
            nc.gpsimd.memset(dum[:, :], 0.0)
            dumo = persist.tile([1, 8], BF16, tag="dumo")
            nc.scalar.activation(dumo[:, :], dum[:, :], EXP)

            onesb = persist.tile([1, DH], BF16, tag="ones")
            nc.gpsimd.memset(onesb[:, :], 1.0)
            warm = persist.tile([P, 512], BF16, tag="warm")
            nc.gpsimd.memset(warm[:, :], 0.5)
            # keep the PE streaming (HAM warm) while the input DMAs land
            wps = psa.tile([P, QC], F32, tag="a")
            for i in range(60):
                nc.tensor.matmul(wps[:, 0:P], lhsT=warm[:, 0:P],
                                 rhs=warm[:, 0:P], start=True, stop=True)

            # ---- persistent SBUF
            xtb = persist.tile([P, 4 * XW], BF16, tag="xtb", name="xtb")
            wb = persist.tile([P, 3 * CT * CH], BF16, tag="wb", name="wb")
            wpb = persist.tile([P, PT * C], BF16, tag="wpb", name="wpb")
            ktb = [persist.tile([P, NK], BF16, tag=f"kt{i}", name=f"kt{i}")
                   for i in range(PT)]
            qtb = [persist.tile([P, NQ], BF16, tag=f"qt{i}", name=f"qt{i}")
                   for i in range(PT)]
            vtb = persist.tile([P, KT * VW], BF16, tag="vtb", name="vtb")
            otb = [persist.tile([P, NQ], BF16, tag=f"ot{i}", name=f"ot{i}")
                   for i in range(PT)]
            y1p = [persist.tile([P, C], BF16, tag=f"y1_{qt}", name=f"y1_{qt}")
                   for qt in range(QC // P)]

            vones = vtb[:, :].rearrange(
                "p (x e) -> p x e", e=DH + 1)[:, :, DH:DH + 1]
            nc.gpsimd.memset(vones, 1.0)

            # ---- contiguous input DMAs over 3 rings, two phases:
            # critical (x chunks 0,1 + K/Q/V weights) stream first; the
            # rest (x chunks 2,3 + Wp) only start once the first K unit has
            # run, so they don't steal HBM bandwidth from the critical path
            dma_copy(nc.sync, xtb[:, 0:XW], xi[:, 0:XW])
            dma_copy(nc.gpsimd, xtb[:, XW:2 * XW], xi[:, XW:2 * XW])
            dma_copy(nc.scalar, wb[:, 0:2 * CT * CH], wkqi[:, :])
            dma_copy(nc.scalar, wb[:, 2 * CT * CH:3 * CT * CH], wvi[:, :])

            # ---- views (x is chunk-major: [c][k][512])
            def xs(k, col, w):
                b = (col // 512) * XW + k * 512 + col % 512
                return xtb[:, b: b + w]

            def wk_v(k, pair):
                b = k * CH + pair * P
                return wb[:, b: b + P]

            def wq_v(k, pair):
                b = CT * CH + k * CH + pair * P
                return wb[:, b: b + P]

            def wvw(k):
                b = 2 * CT * CH + k * CH
                return wb[:, b: b + CH]

            def vv(kt, hl):
                b = kt * VW + hl * (DH + 1)
                return vtb[:, b: b + DH + 1]

            # ---- work units ----
            def kq_unit(is_k, pair, c0, w=512):
                ps = psa.tile([P, QC], F32, tag="a")
                for k in range(CT):
                    for j in range(0, w, 512):
                        jw = min(512, w - j)
                        nc.tensor.matmul(
                            ps[:, j:j + jw],
                            lhsT=(wk_v if is_k else wq_v)(k, pair),
                            rhs=xs(k, c0 + j, jw),
                            start=(k == 0), stop=(k == CT - 1),
                            skip_group_check=True,
                        )
                dst = (ktb if is_k else qtb)[pair]
                nc.vector.tensor_copy(dst[:, c0:c0 + w], ps[:, 0:w])

            def v_unit(kt):
                ps = psa.tile([P, QC], F32, tag="a")
                for k in range(CT):
                    nc.tensor.matmul(
                        ps[:, 0:CH],
                        lhsT=xs(k, kt * P, P),
                        rhs=wvw(k),
                        start=(k == 0), stop=(k == CT - 1),
                    )
                dst = vtb[:, kt * VW:(kt + 1) * VW].rearrange(
                    "p (h e) -> p h e", e=DH + 1)[:, :, 0:DH]
                src = ps[:, 0:CH].rearrange("p (h e) -> p h e", e=DH)
                nc.vector.tensor_copy(dst, src)

            def proj0(qt):
                q0 = qt * P
                ps = psa.tile([P, QC], F32, tag="a")
                for k in range(PT):
                    for c0, csz in ((0, 512), (512, C - 512)):
                        nc.tensor.matmul(
                            ps[:, c0:c0 + csz],
                            lhsT=otb[k][:, q0:q0 + P],
                            rhs=wpb[:, k * C + c0: k * C + c0 + csz],
                            start=(k == 0), stop=(k == PT - 1),
                            skip_group_check=True,
                        )
                yt = ysb.tile([P, C], BF16, tag="y")
                nc.vector.tensor_copy(yt[:, :], ps[:, 0:C])
                nc.sync.dma_start(out=y[q0:q0 + P, :], in_=yt[:, :])

            def proj_pass1(qt):
                q0 = QC + qt * P
                ps = psa.tile([P, QC], F32, tag="a")
                for k in range(2):
                    for c0, csz in ((0, 512), (512, C - 512)):
                        nc.tensor.matmul(
                            ps[:, c0:c0 + csz],
                            lhsT=otb[k][:, q0:q0 + P],
                            rhs=wpb[:, k * C + c0: k * C + c0 + csz],
                            start=(k == 0), stop=(k == 1),
                            skip_group_check=True,
                        )
                nc.vector.tensor_copy(y1p[qt][:, :], ps[:, 0:C])

            def proj_pass2(qt):
                q0 = QC + qt * P
                ps = psa.tile([P, QC], F32, tag="a")
                for c0, csz in ((0, 512), (512, C - 512)):
                    nc.tensor.matmul(
                        ps[:, c0:c0 + csz],
                        lhsT=otb[2][:, q0:q0 + P],
                        rhs=wpb[:, 2 * C + c0: 2 * C + c0 + csz],
                        start=True, stop=True,
                    )
                yt = ysb.tile([P, C], BF16, tag="y")
                nc.vector.tensor_add(yt[:, :], ps[:, 0:C], y1p[qt][:, :])
                nc.sync.dma_start(out=y[q0:q0 + P, :], in_=yt[:, :])

            backlog = deque()

            def pump(n):
                for _ in range(min(n, len(backlog))):
                    backlog.popleft()()

            # ---- attention pieces ----
            state = {"pend": [], "stash": []}

            def make_av(pt, kt, hl, ot):
                def av():
                    for j in range(2):
                        nc.tensor.matmul(
                            ot[:, j * 512:(j + 1) * 512],
                            lhsT=vv(kt, hl),
                            rhs=pt[:, j * 512:(j + 1) * 512],
                            start=(kt == 0), stop=(kt == KT - 1),
                        )
                return av

            def make_stash(ot, osb, den, use_act=False):
                # use_act: final-pair stash only — ScalarE is idle in the
                # tail, so split the drain across ACT and DVE
                def stash():
                    if use_act:
                        nc.scalar.activation(den[:, :], ot[DH:DH + 1, :],
                                             COPY)
                        nc.scalar.activation(osb[:, :], ot[0:DH, :], COPY)
                    else:
                        nc.vector.tensor_copy(den[:, :], ot[DH:DH + 1, :])
                        nc.vector.tensor_copy(osb[:, :], ot[0:DH, :])
                return stash

            def make_norm(pair, hr, qc, osb, den):
                box = {}

                def pre():
                    rec = nsb.tile([1, QC], F32, tag="rec", bufs=2)
                    nc.vector.reciprocal_approx_fast(out=rec[:, :],
                                                     in_=den[:, :])
                    recb = nsb.tile([1, QC], BF16, tag="recb", bufs=2)
                    nc.vector.tensor_copy(recb[:, :], rec[:, :])
                    box["recb"] = recb

                def mm():
                    recb = box["recb"]
                    rb = psa.tile([P, QC], F32, tag="a")
                    nc.tensor.matmul(
                        rb[0:DH, 0:512], lhsT=onesb[0:1, :],
                        rhs=recb[0:1, 0:512],
                        start=True, stop=True, tile_position=(0, 0),
                    )
                    nc.tensor.matmul(
                        rb[DH:P, 512:QC], lhsT=onesb[0:1, :],
                        rhs=recb[0:1, 512:QC],
                        start=True, stop=True, tile_position=(0, DH),
                    )
                    nc.vector.tensor_mul(
                        otb[pair][hr:hr + DH, qc * QC:qc * QC + 512],
                        osb[:, 0:512], rb[0:DH, 0:512],
                    )
                    nc.vector.tensor_mul(
                        otb[pair][hr:hr + DH, qc * QC + 512:(qc + 1) * QC],
                        osb[:, 512:QC], rb[DH:P, 512:QC],
                    )
                return pre, mm

            # ---- one (qc, pair) block ----
            def block(qc, pair, pump_plan, norms, last=False):
                ots = [psb.tile([DH + 1, QC], F32, tag="b", name=f"ot{hh}")
                       for hh in range(2)]
                for kt in range(KT):
                    sts = []
                    for hh in range(2):
                        st = psa.tile([P, QC], F32, tag="a")
                        sts.append(st)
                    # j-inner: consecutive matmuls share the loaded K tile
                    # (every weight switch costs ~95ns of PE stream)
                    for hh in range(2):
                        hr = hh * DH
                        for j in range(2):
                            nc.tensor.matmul(
                                sts[hh][:, j * 512:(j + 1) * 512],
                                lhsT=ktb[pair][hr:hr + DH,
                                               kt * P:(kt + 1) * P],
                                rhs=qtb[pair][hr:hr + DH,
                                              qc * QC + j * 512:
                                              qc * QC + (j + 1) * 512],
                                start=True, stop=True,
                                tile_position=(hr, 0),
                            )
                    for fn in state["pend"]:
                        fn()
                    state["pend"] = []
                    for fn in state["stash"]:
                        fn()
                    state["stash"] = []
                    for hh in range(2):
                        pt = pP.tile([P, QC], BF16, tag="p")
                        nc.scalar.activation(pt[:, :], sts[hh][:, :], EXP,
                                             scale=SCALE)
                        state["pend"].append(
                            make_av(pt, kt, pair * 2 + hh, ots[hh]))
                    # pump/norm AFTER the exps (psa ring-reuse WAR order)
                    if kt == 2 and norms:
                        norms[0][0]()
                    elif kt == 3 and norms:
                        norms[1][0]()
                    elif kt == 5 and norms:
                        norms[0][1]()
                    elif kt == 7 and norms:
                        norms[1][1]()
                    else:
                        pump(pump_plan[kt])
                out_norms = []
                for hh in range(2):
                    osb = nsb.tile([DH, QC], F32, tag="osb", bufs=4,
                                   name=f"osb{hh}")
                    den = nsb.tile([1, QC], F32, tag="den", bufs=4,
                                   name=f"den{hh}")
                    state["stash"].append(
                        make_stash(ots[hh], osb, den, use_act=(last and hh)))
                    out_norms.append(make_norm(pair, hh * DH, qc, osb, den))
                return out_norms

            # ---- prelude compute: minimum for the first exp ----
            kq_unit(True, 0, 0, 128)     # K pair0 keys 0-127
            # deferred-DMA trigger: a tiny gpsimd copy that depends on the
            # first K unit gates the non-critical DMAs behind it
            trig = persist.tile([1, 8], BF16, tag="trig")
            nc.gpsimd.tensor_copy(trig[:, :], ktb[0][0:1, 0:8])
            for c in (2, 3):
                dma_copy(nc.gpsimd, xtb[:, c * XW:(c + 1) * XW],
                         xi[:, c * XW:(c + 1) * XW])
            dma_copy(nc.gpsimd, wpb[:, :], wpi[:, :])
            kq_unit(False, 0, 0, 1024)   # Q pair0 queries 0-1023
            kq_unit(True, 0, 512)        # K pair0 keys 512-1023 (chunk 1)

            # ---- backlog, dependency order; V(kt) just-in-time ----
            def V(kt):
                backlog.append(lambda kt=kt: v_unit(kt))

            def KQ(is_k, pair, c0, w=512):
                backlog.append(
                    lambda i=is_k, p=pair, c=c0, w=w: kq_unit(i, p, c, w))

            KQ(True, 0, 128, 384)
            V(0); V(1); V(2); V(3); V(4); V(5)
            KQ(True, 0, 1024, 1024)
            V(6); V(7); V(8)
            KQ(False, 0, 1024, 1024)
            for kt in range(9, KT):
                V(kt)
            for pair in (1, 2):
                for half in range(2):
                    KQ(True, pair, half * 1024, 1024)
                for half in range(2):
                    KQ(False, pair, half * 1024, 1024)

            # ---- blocks, pair-major; block b runs block b-1's norms ----
            plan0 = [2, 1, 1, 1, 1, 2, 1, 2, 1, 1, 2, 1, 1, 1, 1, 1]
            plan_b1 = [1, 0, 0, 0, 1, 0, 1, 0, 1, 0, 0, 0, 0, 0, 0, 0]
            plan_b2 = [1, 0, 0, 0, 1, 0, 0, 0, 0, 0, 0, 0, 0, 0, 0, 0]
            plan_late = [0] * 8 + [1] * 8
            nrm = block(0, 0, plan0, None)
            nrm = block(1, 0, plan_b1, nrm)
            nrm = block(0, 1, plan_b2, nrm)
            nrm = block(1, 1, plan_b2, nrm)
            for qt in range(QC // P):
                backlog.append(lambda qt=qt: proj_pass1(qt))
            nrm = block(0, 2, plan_late, nrm)
            for qt in range(QC // P):
                backlog.append(lambda qt=qt: proj0(qt))
            nrm = block(1, 2, plan_late, nrm, last=True)

            # ---- tail ----
            for fn in state["pend"]:
                fn()
            state["pend"] = []
            for fn in state["stash"]:
                fn()
            state["stash"] = []
            pump(len(backlog))
            # keep the PE warm through the final norm's DVE chain
            wtl = psa.tile([P, QC], F32, tag="a")
            for i in range(45):
                nc.tensor.matmul(wtl[:, 0:P], lhsT=warm[:, 0:P],
                                 rhs=warm[:, 0:P], start=True, stop=True)
            nrm[0][0]()
            nrm[1][0]()
            nrm[0][1]()
            nrm[1][1]()
            for qt in range(QC // P):
                proj_pass2(qt)
            if debug:
                for i in range(PT):
                    tmp = ysb.tile([P, NK], F32, tag="dbg", bufs=2)
                    nc.vector.tensor_copy(tmp[:, :], ktb[i][:, :])
                    nc.sync.dma_start(out=dkt[i * P:(i + 1) * P, :],
                                      in_=tmp[:, :])
                    tmp = ysb.tile([P, NQ], F32, tag="dbg", bufs=2)
                    nc.vector.tensor_copy(tmp[:, :], qtb[i][:, :])
                    nc.sync.dma_start(out=dqt[i * P:(i + 1) * P, :],
                                      in_=tmp[:, :])
                    tmp = ysb.tile([P, NQ], F32, tag="dbg", bufs=2)
                    nc.vector.tensor_copy(tmp[:, :], otb[i][:, :])
                    nc.sync.dma_start(out=dot[i * P:(i + 1) * P, :],
                                      in_=tmp[:, :])
                tmp = ysb.tile([P, KT * VW], F32, tag="dbgv", bufs=1)
                nc.vector.tensor_copy(tmp[:, :], vtb[:, :])
                nc.sync.dma_start(out=dvt[:, :], in_=tmp[:, :])

    nc.compile()
    return nc


def _get_prog() -> bass.Bass:
    global _PROG
    if _PROG is None:
        _PROG = _build_program()
    return _PROG


def _swizzle(mat, inner):
    # [CT*128, inner] channel-major -> [128, CT*inner] SBUF image
    kt = mat.shape[0] // P
    return np.ascontiguousarray(
        mat.reshape(kt, P, inner).transpose(1, 0, 2).reshape(P, kt * inner))


def kernel(x, Wq, Wk, Wv, Wp, bp):
    global LAST_RESULT
    x = np.asarray(x, np.float32)
    Wq = np.asarray(Wq, np.float32)
    Wk = np.asarray(Wk, np.float32)
    Wv = np.asarray(Wv, np.float32)
    Wp = np.asarray(Wp, np.float32)
    bp = np.asarray(bp, np.float32)

    B, N, _ = x.shape
    # x image: [p, c-chunk, k, 512] chunk-major flat
    xis = []
    for b in range(B):
        xT = np.ascontiguousarray(x[b].T)              # [768, 2048]
        img = xT.reshape(CT, P, 4, 512).transpose(1, 2, 0, 3)
        xis.append(np.ascontiguousarray(
            img.reshape(P, 4 * XW)).astype(BF16_NP))
    wkq_h, wv_h, wp_h = [], [], []
    for hh in range(2):
        r = slice(hh * CH, (hh + 1) * CH)
        kimg = _swizzle(Wk[r].T, CH)
        qimg = _swizzle(Wq[r].T, CH)
        vimg = _swizzle(Wv[r].T, CH)
        wkq_h.append(np.concatenate([kimg, qimg], axis=1).astype(BF16_NP))
        wv_h.append(vimg.astype(BF16_NP))
        wp_h.append(_swizzle(Wp.T[r], C).astype(BF16_NP))

    in_maps = []
    for core in range(8):
        b, hh = core // 2, core % 2
        in_maps.append({
            "xi": xis[b],
            "wkqi": wkq_h[hh],
            "wvi": wv_h[hh],
            "wpi": wp_h[hh],
        })

    res = run_bass_kernel_spmd(
        _get_prog(), in_maps, core_ids=list(range(8)),
        trace=bool(os.environ.get("BASS_TRACE")),
    )
    LAST_RESULT = res

    out = np.empty((B, N, C), np.float32)
    for b in range(B):
        out[b] = (res.results[2 * b]["y"].astype(np.float32)
                  + res.results[2 * b + 1]["y"].astype(np.float32) + bp)
    return out
